# revision 1
# baseline (speedup 1.0000x reference)
"""ContactDiffusion kernel for 8 Trainium2 NeuronCores.

Computes, for N=8192, d=64:
    a[i,j] = 0.5*(alpha[i]+alpha[j])
    K = (D + 1e-8)**(-a) * exp(-D/12)     (diag zeroed)
    K = K / (K.sum(-1, keepdims=True) + 1e-8)
    returns (K @ latent, K)

Math on device:  K0 = exp(-((0.5*alpha_j + 0.5*alpha_i) * ln(D + 1e-8) + D/12))
Diagonal zeroing is done by host-poisoning diag(D_shard) to 1e10 so that
exp(-(...)-8.3e8) == 0 exactly.

Sharding: rows of D/K/out are split into 8 blocks of 1024 rows (one per
core); latent is replicated, so K @ latent needs no cross-device reduction.

Per-core program (straight-line, Tile-scheduled):
  per row block rb (8 x [128, 8192]):
    per column chunk (4 x [128, 2048]):
      DMA D chunk; ACT: L = ln(D+1e-8);
      DVE: L = (B + 0.5*alpha_i) * L        (scalar_tensor_tensor, B = bcast 0.5*alpha_j)
      DVE: S = D*(1/12) + L                 (scalar_tensor_tensor, in place over D)
      ACT: K0 chunk = exp(-S)
      PE : transpose K0 128x128 blocks -> PSUM (batched 8 per copy)
      ACT/DVE: copy PSUM -> SBUF
      PE : opsum[128,65] += K0_block^T.T @ [latent_block | ones]
    inv = 1/(opsum[:,64] + 1e-8)
    out rows  = opsum[:,:64] * inv  -> DMA
    K rows    = K0 * inv (GPSIMD)   -> DMA
"""

import numpy as np

N = 8192
DLAT = 64
DL1 = DLAT + 1  # latent columns + ones column (for row sums via PE)
NCORES = 8
ROWS = N // NCORES  # 1024 rows per core
P = 128  # partitions / row block size
NRB = ROWS // P  # 8 row blocks per core
CHUNK = 2048  # column chunk for the elementwise pipeline
NCH = N // CHUNK
TB = 1024  # transpose batch width (8 transposes per PSUM->SBUF copy)
NJB = N // P  # 64 column blocks of 128 for the matmul

DIAG_POISON = 1.0e10

# PSUM->SBUF copy engine pattern: out of every 8 copies, this many go to the
# scalar (ACT) engine, the rest to vector (DVE).
TCOPY_ACT_OF_8 = 5

_CACHE = {}


def _build_nc():
    import concourse.bacc as bacc
    import concourse.mybir as mybir
    from concourse.tile import TileContext

    f32 = mybir.dt.float32
    AF = mybir.ActivationFunctionType
    OP = mybir.AluOpType

    nc = bacc.Bacc(target_bir_lowering=False)

    d = nc.dram_tensor("d_shard", [ROWS, N], f32, kind="ExternalInput")
    bb = nc.dram_tensor("b_bcast", [P, N], f32, kind="ExternalInput")
    latb = nc.dram_tensor("lat_blocks", [P, NJB, DL1], f32, kind="ExternalInput")
    ar = nc.dram_tensor("alpha_rows", [P, NRB], f32, kind="ExternalInput")
    ident = nc.dram_tensor("ident", [P, P], f32, kind="ExternalInput")
    consts = nc.dram_tensor("consts", [P, 2], f32, kind="ExternalInput")
    k_out = nc.dram_tensor("k_out", [ROWS, N], f32, kind="ExternalOutput")
    o_out = nc.dram_tensor("o_out", [ROWS, DLAT], f32, kind="ExternalOutput")

    with TileContext(nc) as tc:
        with (
            tc.tile_pool(name="constp", bufs=1) as constp,
            tc.tile_pool(name="workp", bufs=3) as workp,
            tc.tile_pool(name="kpool", bufs=2) as kpool,
            tc.tile_pool(name="tsbp", bufs=3) as tsbp,
            tc.tile_pool(name="smallp", bufs=2) as smallp,
            tc.tile_pool(name="psum_o", bufs=2, space="PSUM") as psum_o,
            tc.tile_pool(name="psum_t", bufs=2, space="PSUM") as psum_t,
        ):
            b_sb = constp.tile([P, N], f32, name="b_sb")
            nc.sync.dma_start(out=b_sb, in_=bb[:, :])
            lat_sb = constp.tile([P, NJB, DL1], f32, name="lat_sb")
            nc.sync.dma_start(out=lat_sb, in_=latb[:, :, :])
            ar_sb = constp.tile([P, NRB], f32, name="ar_sb")
            nc.sync.dma_start(out=ar_sb, in_=ar[:, :])
            id_sb = constp.tile([P, P], f32, name="id_sb")
            nc.sync.dma_start(out=id_sb, in_=ident[:, :])
            cst_sb = constp.tile([P, 2], f32, name="cst_sb")
            nc.sync.dma_start(out=cst_sb, in_=consts[:, :])
            eps_ap = cst_sb[:, 0:1]
            zero_ap = cst_sb[:, 1:2]

            tcopy_idx = 0
            for rb in range(NRB):
                r0 = rb * P
                k0 = kpool.tile([P, N], f32, name="k0", tag="k0")
                opsum = psum_o.tile([P, DL1], f32, name="opsum", tag="op")
                for ch in range(NCH):
                    c0 = ch * CHUNK
                    dt_ = workp.tile([P, CHUNK], f32, name="dt", tag="d")
                    nc.sync.dma_start(out=dt_, in_=d[r0 : r0 + P, c0 : c0 + CHUNK])
                    lt = workp.tile([P, CHUNK], f32, name="lt", tag="l")
                    # L = ln(D + 1e-8)
                    nc.scalar.activation(lt, dt_, AF.Ln, bias=eps_ap, scale=1.0)
                    # L = (B + 0.5*alpha_i) * L     (in place over L)
                    nc.vector.scalar_tensor_tensor(
                        lt,
                        b_sb[:, c0 : c0 + CHUNK],
                        ar_sb[:, rb : rb + 1],
                        lt,
                        op0=OP.add,
                        op1=OP.mult,
                    )
                    # S = D*(1/12) + L              (in place over D)
                    nc.vector.scalar_tensor_tensor(
                        dt_, dt_, 1.0 / 12.0, lt, op0=OP.mult, op1=OP.add
                    )
                    # K0 = exp(-S)
                    nc.scalar.activation(
                        k0[:, c0 : c0 + CHUNK], dt_, AF.Exp, bias=zero_ap, scale=-1.0
                    )
                    for half in range(CHUNK // TB):
                        h0 = c0 + half * TB
                        tps = psum_t.tile([P, TB], f32, name="tps", tag="tp")
                        for b8 in range(TB // P):
                            j0 = h0 + b8 * P
                            nc.tensor.transpose(
                                tps[:, b8 * P : (b8 + 1) * P], k0[:, j0 : j0 + P], id_sb
                            )
                        tsb = tsbp.tile([P, TB], f32, name="tsb", tag="tsb")
                        if tcopy_idx % 8 < TCOPY_ACT_OF_8:
                            nc.scalar.copy(tsb, tps)
                        else:
                            nc.vector.tensor_copy(tsb, tps)
                        tcopy_idx += 1
                        for b8 in range(TB // P):
                            jb = h0 // P + b8
                            nc.tensor.matmul(
                                opsum,
                                tsb[:, b8 * P : (b8 + 1) * P],
                                lat_sb[:, jb, :],
                                start=(jb == 0),
                                stop=(jb == NJB - 1),
                            )
                # inv = 1 / (rowsum + 1e-8)
                inv = smallp.tile([P, 1], f32, name="inv", tag="inv")
                nc.vector.tensor_scalar_add(inv, opsum[:, DLAT:DL1], 1.0e-8)
                nc.vector.reciprocal(inv, inv)
                osb = smallp.tile([P, DLAT], f32, name="osb", tag="osb")
                nc.vector.tensor_scalar_mul(osb, opsum[:, 0:DLAT], inv)
                nc.sync.dma_start(out=o_out[r0 : r0 + P, :], in_=osb)
                for ch in range(NCH):
                    c0 = ch * CHUNK
                    nc.gpsimd.tensor_scalar_mul(
                        k0[:, c0 : c0 + CHUNK], k0[:, c0 : c0 + CHUNK], inv
                    )
                    nc.sync.dma_start(
                        out=k_out[r0 : r0 + P, c0 : c0 + CHUNK],
                        in_=k0[:, c0 : c0 + CHUNK],
                    )

    nc.compile()
    return nc


def get_nc():
    if "nc" not in _CACHE:
        _CACHE["nc"] = _build_nc()
    return _CACHE["nc"]


def prepare_in_maps(latent, D, alpha):
    latent = np.asarray(latent, dtype=np.float32)
    D = np.asarray(D, dtype=np.float32)
    alpha = np.asarray(alpha, dtype=np.float32)

    lat_aug = np.concatenate([latent, np.ones((N, 1), np.float32)], axis=1)  # [N, 65]
    # lat_blocks[p, jb, c] = lat_aug[jb*128 + p, c]
    lat_blocks = np.ascontiguousarray(
        lat_aug.reshape(NJB, P, DL1).transpose(1, 0, 2)
    )
    b_bcast = np.ascontiguousarray(
        np.broadcast_to(0.5 * alpha[None, :], (P, N))
    )
    ident = np.eye(P, dtype=np.float32)
    consts = np.zeros((P, 2), np.float32)
    consts[:, 0] = 1.0e-8

    in_maps = []
    for c in range(NCORES):
        rows = slice(c * ROWS, (c + 1) * ROWS)
        d_shard = D[rows].copy()
        d_shard[np.arange(ROWS), np.arange(c * ROWS, (c + 1) * ROWS)] = DIAG_POISON
        # alpha_rows[p, rb] = 0.5*alpha[c*ROWS + rb*128 + p]
        alpha_rows = np.ascontiguousarray(
            (0.5 * alpha[rows]).reshape(NRB, P).T
        )
        in_maps.append(
            {
                "d_shard": d_shard,
                "b_bcast": b_bcast,
                "lat_blocks": lat_blocks,
                "alpha_rows": alpha_rows,
                "ident": ident,
                "consts": consts,
            }
        )
    return in_maps


def assemble(results):
    K = np.concatenate([r["k_out"] for r in results], axis=0)
    out = np.concatenate([r["o_out"] for r in results], axis=0)
    return out, K


def kernel(latent, D, alpha):
    from concourse.bass_utils import run_bass_kernel_spmd

    nc = get_nc()
    in_maps = prepare_in_maps(latent, D, alpha)
    res = run_bass_kernel_spmd(nc, in_maps, core_ids=list(range(NCORES)))
    return assemble(res.results)


# revision 21
# speedup vs baseline: 27.8655x; 27.8655x over previous
"""ContactDiffusion kernel for 8 Trainium2 NeuronCores.

Computes, for N=8192, d=64:
    a[i,j] = 0.5*(alpha[i]+alpha[j])
    K = (D + 1e-8)**(-a) * exp(-D/12)     (diag zeroed)
    K = K / (K.sum(-1, keepdims=True) + 1e-8)
    returns (K @ latent, K)

Math on device:  K0 = exp(-((0.5*alpha_j + 0.5*alpha_i) * ln(D + 1e-8) + D/12))
Diagonal zeroing is done by host-poisoning diag(D_shard) to 1e10 so that
exp(-(...)-8.3e8) == 0 exactly.

Sharding: rows of D/K/out are split into 8 blocks of 1024 rows (one per
core); latent is replicated, so K @ latent needs no cross-device reduction.

Per-core program (straight-line, Tile-scheduled):
  per row block rb (8 x [128, 8192]):
    per column chunk (4 x [128, 2048]):
      DMA D chunk; ACT: L = ln(D+1e-8)
      DVE: L = (B + 0.5*alpha_i) * L     (scalar_tensor_tensor, B = bcast 0.5*alpha_j)
      DVE: S = D*(1/12) + L              (scalar_tensor_tensor, in place over D)
      ACT: K0 chunk = exp(-S), fp32 row-sum partials free via accum_out
      PE : transpose K0 128x128 blocks -> PSUM (fp32, batched 8 per copy)
      ACT/DVE (auto): copy PSUM -> SBUF, converting to bf16
      PE : opsum[128,64] += K0_block^T.T @ latent_block   (bf16, FWL)
    inv = 1/(sum of row-sum partials + 1e-8)
    out rows = opsum * inv -> DMA
    K rows   = K0 * inv (DVE tensor_scalar, 2x mode) -> DMA on the ACT HWDGE
    ring (loads use the SP ring, so stores never queue behind loads)
"""

import numpy as np

N = 8192
DLAT = 64
DL1 = DLAT + 1  # latent columns + ones column (for row sums via PE)
NCORES = 8
ROWS = N // NCORES  # 1024 rows per core
P = 128  # partitions / row block size
NRB = ROWS // P  # 8 row blocks per core
CHUNK = 2048  # column chunk for the elementwise pipeline
NCH = N // CHUNK
TB = 1024  # transpose batch width (8 transposes per PSUM->SBUF copy)
NJB = N // P  # 64 column blocks of 128 for the matmul

DIAG_POISON = 1.0e10

_CACHE = {}


def _compile_with_shared_act_set(nc):
    """Force all activations (Ln/Exp/Copy) onto the single table set that
    contains them all, so bacc inserts one table load instead of thrashing
    between per-function sets (~2.7us per reload on the ACT engine)."""
    import concourse.bacc as bacc
    import concourse.mybir as mybir

    AF = mybir.ActivationFunctionType
    orig = bacc.get_activation_tables
    shared = {AF.Ln, AF.Exp, AF.Copy}

    def patched(arch):
        tabs = orig(arch)
        keep = None
        for name, funcs in tabs.items():
            if shared <= funcs:
                keep = name
                break
        if keep is None:
            return tabs
        return {
            name: (funcs if name == keep else funcs - shared)
            for name, funcs in tabs.items()
        }

    bacc.get_activation_tables = patched
    try:
        nc.compile()
    finally:
        bacc.get_activation_tables = orig


def _build_nc(repeat=1, do_elem=True, do_pe=True, do_scale=True):
    import concourse.bacc as bacc
    import concourse.mybir as mybir
    from concourse.tile import TileContext

    f32 = mybir.dt.float32
    AF = mybir.ActivationFunctionType
    OP = mybir.AluOpType

    nc = bacc.Bacc(target_bir_lowering=False)

    bf16 = mybir.dt.bfloat16
    d = nc.dram_tensor("d_shard", [ROWS, N], f32, kind="ExternalInput")
    bb = nc.dram_tensor("b_bcast", [P, N], f32, kind="ExternalInput")
    latb = nc.dram_tensor("lat_blocks", [P, NJB, DL1], bf16, kind="ExternalInput")
    ar = nc.dram_tensor("alpha_rows", [P, NRB], f32, kind="ExternalInput")
    ident = nc.dram_tensor("ident", [P, P], f32, kind="ExternalInput")
    consts = nc.dram_tensor("consts", [P, 2], f32, kind="ExternalInput")
    k_out = nc.dram_tensor("k_out", [ROWS, N], f32, kind="ExternalOutput")
    o_out = nc.dram_tensor("o_out", [ROWS, DLAT], f32, kind="ExternalOutput")

    with TileContext(nc) as tc:
        with (
            tc.tile_pool(name="constp", bufs=1) as constp,
            tc.tile_pool(name="workp", bufs=4) as workp,
            tc.tile_pool(name="kpool", bufs=2) as kpool,
            tc.tile_pool(name="tsbp", bufs=3) as tsbp,
            tc.tile_pool(name="smallp", bufs=2) as smallp,
            tc.tile_pool(name="psum_o", bufs=2, space="PSUM") as psum_o,
            tc.tile_pool(name="psum_t", bufs=3, space="PSUM") as psum_t,
        ):
            b_sb = constp.tile([P, N], f32, name="b_sb")
            nc.sync.dma_start(out=b_sb, in_=bb[:, :])
            lat_sb = constp.tile([P, NJB, DL1], bf16, name="lat_sb")
            nc.sync.dma_start(out=lat_sb, in_=latb[:, :, :])
            ar_sb = constp.tile([P, NRB], f32, name="ar_sb")
            nc.sync.dma_start(out=ar_sb, in_=ar[:, :])
            id_sb = constp.tile([P, P], f32, name="id_sb")
            nc.sync.dma_start(out=id_sb, in_=ident[:, :])
            cst_sb = constp.tile([P, 2], f32, name="cst_sb")
            nc.sync.dma_start(out=cst_sb, in_=consts[:, :])
            eps_ap = cst_sb[:, 0:1]
            zero_ap = cst_sb[:, 1:2]

            for rb in [r for _ in range(repeat) for r in range(NRB)]:
                r0 = rb * P
                k0 = kpool.tile([P, N], f32, name="k0", tag="k0")
                opsum = (
                    psum_o.tile([P, DLAT], f32, name="opsum", tag="op")
                    if do_pe
                    else None
                )
                rs_parts = []
                for ch in range(NCH):
                    c0 = ch * CHUNK
                    if do_elem:
                        dt_ = workp.tile([P, CHUNK], f32, name="dt", tag="d")
                        nc.sync.dma_start(
                            out=dt_, in_=d[r0 : r0 + P, c0 : c0 + CHUNK]
                        )
                    else:
                        nc.sync.dma_start(
                            out=k0[:, c0 : c0 + CHUNK],
                            in_=d[r0 : r0 + P, c0 : c0 + CHUNK],
                        )
                    if do_elem:
                        lt = workp.tile([P, CHUNK], f32, name="lt", tag="l")
                        # L = ln(D + 1e-8)
                        nc.scalar.activation(lt, dt_, AF.Ln, bias=eps_ap, scale=1.0)
                        # L = (B + 0.5*alpha_i) * L     (in place over L)
                        nc.vector.scalar_tensor_tensor(
                            lt,
                            b_sb[:, c0 : c0 + CHUNK],
                            ar_sb[:, rb : rb + 1],
                            lt,
                            op0=OP.add,
                            op1=OP.mult,
                        )
                        # S = D*(1/12) + L              (in place over D)
                        nc.vector.scalar_tensor_tensor(
                            dt_, dt_, 1.0 / 12.0, lt, op0=OP.mult, op1=OP.add
                        )
                        # K0 = exp(-S); accum_out gives the fp32 row-sum free
                        rs = smallp.tile([P, 1], f32, name="rs", tag="rs", bufs=8)
                        rs_parts.append(rs)
                        nc.scalar.activation(
                            k0[:, c0 : c0 + CHUNK],
                            dt_,
                            AF.Exp,
                            bias=zero_ap,
                            scale=-1.0,
                            accum_out=rs,
                        )
                    for half in range(CHUNK // TB) if do_pe else []:
                        h0 = c0 + half * TB
                        tps = psum_t.tile([P, TB], f32, name="tps", tag="tp")
                        for b8 in range(TB // P):
                            j0 = h0 + b8 * P
                            nc.tensor.transpose(
                                tps[:, b8 * P : (b8 + 1) * P], k0[:, j0 : j0 + P], id_sb
                            )
                        tsb = tsbp.tile([P, TB], bf16, name="tsb", tag="tsb")
                        nc.any.tensor_copy(tsb, tps)
                        for b8 in range(TB // P):
                            jb = h0 // P + b8
                            nc.tensor.matmul(
                                opsum,
                                tsb[:, b8 * P : (b8 + 1) * P],
                                lat_sb[:, jb, 0:DLAT],
                                start=(jb == 0),
                                stop=(jb == NJB - 1),
                            )
                # inv = 1 / (rowsum + 1e-8)
                if do_elem and (do_pe or do_scale):
                    inv = smallp.tile([P, 1], f32, name="inv", tag="inv")
                    nc.vector.tensor_tensor(
                        inv, rs_parts[0], rs_parts[1], op=OP.add
                    )
                    for rs in rs_parts[2:]:
                        nc.vector.tensor_tensor(inv, inv, rs, op=OP.add)
                    nc.vector.tensor_scalar_add(inv, inv, 1.0e-8)
                    nc.vector.reciprocal(inv, inv)
                elif do_scale:
                    inv = smallp.tile([P, 1], f32, name="inv", tag="inv")
                    nc.vector.memset(inv, 1.0)
                if do_pe:
                    osb = smallp.tile([P, DLAT], f32, name="osb", tag="osb")
                    nc.vector.tensor_scalar_mul(osb, opsum[:, 0:DLAT], inv)
                    nc.sync.dma_start(out=o_out[r0 : r0 + P, :], in_=osb)
                for ch in range(NCH):
                    c0 = ch * CHUNK
                    if do_scale:
                        nc.vector.tensor_scalar_mul(
                            k0[:, c0 : c0 + CHUNK], k0[:, c0 : c0 + CHUNK], inv
                        )
                    nc.scalar.dma_start(
                        out=k_out[r0 : r0 + P, c0 : c0 + CHUNK],
                        in_=k0[:, c0 : c0 + CHUNK],
                    )

    _compile_with_shared_act_set(nc)
    return nc


def get_nc():
    if "nc" not in _CACHE:
        _CACHE["nc"] = _build_nc()
    return _CACHE["nc"]


def prepare_in_maps(latent, D, alpha):
    latent = np.asarray(latent, dtype=np.float32)
    D = np.asarray(D, dtype=np.float32)
    alpha = np.asarray(alpha, dtype=np.float32)

    import ml_dtypes

    lat_aug = np.concatenate([latent, np.ones((N, 1), np.float32)], axis=1)  # [N, 65]
    # lat_blocks[p, jb, c] = lat_aug[jb*128 + p, c]
    lat_blocks = np.ascontiguousarray(
        lat_aug.reshape(NJB, P, DL1).transpose(1, 0, 2)
    ).astype(ml_dtypes.bfloat16)
    b_bcast = np.ascontiguousarray(
        np.broadcast_to(0.5 * alpha[None, :], (P, N))
    )
    ident = np.eye(P, dtype=np.float32)
    consts = np.zeros((P, 2), np.float32)
    consts[:, 0] = 1.0e-8

    in_maps = []
    for c in range(NCORES):
        rows = slice(c * ROWS, (c + 1) * ROWS)
        d_shard = D[rows].copy()
        d_shard[np.arange(ROWS), np.arange(c * ROWS, (c + 1) * ROWS)] = DIAG_POISON
        # alpha_rows[p, rb] = 0.5*alpha[c*ROWS + rb*128 + p]
        alpha_rows = np.ascontiguousarray(
            (0.5 * alpha[rows]).reshape(NRB, P).T
        )
        in_maps.append(
            {
                "d_shard": d_shard,
                "b_bcast": b_bcast,
                "lat_blocks": lat_blocks,
                "alpha_rows": alpha_rows,
                "ident": ident,
                "consts": consts,
            }
        )
    return in_maps


def assemble(results):
    K = np.concatenate([r["k_out"] for r in results], axis=0)
    out = np.concatenate([r["o_out"] for r in results], axis=0)
    return out, K


def kernel(latent, D, alpha):
    from concourse.bass_utils import run_bass_kernel_spmd

    nc = get_nc()
    in_maps = prepare_in_maps(latent, D, alpha)
    res = run_bass_kernel_spmd(nc, in_maps, core_ids=list(range(NCORES)))
    return assemble(res.results)


# revision 31
# speedup vs baseline: 34.1329x; 1.2249x over previous
"""ContactDiffusion kernel for 8 Trainium2 NeuronCores.

Computes, for N=8192, d=64:
    a[i,j] = 0.5*(alpha[i]+alpha[j])
    K = (D + 1e-8)**(-a) * exp(-D/12)     (diag zeroed)
    K = K / (K.sum(-1, keepdims=True) + 1e-8)
    returns (K @ latent, K)

Math on device:  K0 = exp(-((0.5*alpha_j + 0.5*alpha_i) * ln(D + 1e-8) + D/12))
Diagonal zeroing is done by host-poisoning diag(D_shard) to 1e10 so that
exp(-(...)-8.3e8) == 0 exactly.

Sharding: rows of D/K/out are split into 8 blocks of 1024 rows (one per
core); latent is replicated, so K @ latent needs no cross-device reduction.

Per-core program (straight-line, Tile-scheduled):
  per row block rb (8 x [128, 8192]):
    per column chunk (4 x [128, 2048]):
      DMA D chunk; ACT: L = ln(D+1e-8)
      DVE: L = (B + 0.5*alpha_i) * L     (scalar_tensor_tensor, B = bcast 0.5*alpha_j)
      DVE: S = D*(1/12) + L              (scalar_tensor_tensor, in place over D)
      ACT: K0 chunk = exp(-S), fp32 row-sum partials free via accum_out
      PE : transpose K0 128x128 blocks -> PSUM (fp32, batched 8 per copy)
      ACT/DVE (auto): copy PSUM -> SBUF, converting to bf16
      PE : opsum[128,64] += K0_block^T.T @ latent_block   (bf16, FWL)
    inv = 1/(sum of row-sum partials + 1e-8)
    out rows = opsum * inv -> DMA
    K rows   = K0 * inv (DVE tensor_scalar, 2x mode) -> DMA on the ACT HWDGE
    ring (loads use the SP ring, so stores never queue behind loads)
"""

import numpy as np

N = 8192
DLAT = 64
DL1 = DLAT + 1  # latent columns + ones column (for row sums via PE)
NCORES = 8
ROWS = N // NCORES  # 1024 rows per core
P = 128  # partitions / row block size
NRB = ROWS // P  # 8 row blocks per core
CHUNK = 2048  # column chunk for the elementwise pipeline
NCH = N // CHUNK
TB = 1024  # transpose batch width (8 transposes per PSUM->SBUF copy)
NJB = N // P  # 64 column blocks of 128 for the matmul

DIAG_POISON = 1.0e10

_CACHE = {}


def _compile_with_shared_act_set(nc):
    """Force all activations (Ln/Exp/Copy) onto the single table set that
    contains them all, so bacc inserts one table load instead of thrashing
    between per-function sets (~2.7us per reload on the ACT engine)."""
    import concourse.bacc as bacc
    import concourse.mybir as mybir

    AF = mybir.ActivationFunctionType
    orig = bacc.get_activation_tables
    shared = {AF.Ln, AF.Exp, AF.Copy}

    def patched(arch):
        tabs = orig(arch)
        keep = None
        for name, funcs in tabs.items():
            if shared <= funcs:
                keep = name
                break
        if keep is None:
            return tabs
        return {
            name: (funcs if name == keep else funcs - shared)
            for name, funcs in tabs.items()
        }

    bacc.get_activation_tables = patched
    try:
        nc.compile()
    finally:
        bacc.get_activation_tables = orig


def _build_nc(
    repeat=1,
    do_elem=True,
    do_pe=True,
    do_scale=True,
    chunk=CHUNK,
    work_bufs=4,
    stt2_pool_every=0,
):
    import concourse.bacc as bacc
    import concourse.mybir as mybir
    from concourse.tile import TileContext

    nch = N // chunk

    f32 = mybir.dt.float32
    AF = mybir.ActivationFunctionType
    OP = mybir.AluOpType

    nc = bacc.Bacc(target_bir_lowering=False)

    bf16 = mybir.dt.bfloat16
    d = nc.dram_tensor("d_shard", [ROWS, N], f32, kind="ExternalInput")
    bb = nc.dram_tensor("b_bcast", [P, N], f32, kind="ExternalInput")
    latb = nc.dram_tensor("lat_blocks", [P, NJB, DL1], bf16, kind="ExternalInput")
    ar = nc.dram_tensor("alpha_rows", [P, NRB], f32, kind="ExternalInput")
    ident = nc.dram_tensor("ident", [P, P], f32, kind="ExternalInput")
    consts = nc.dram_tensor("consts", [P, 2], f32, kind="ExternalInput")
    k_out = nc.dram_tensor("k_out", [ROWS, N], f32, kind="ExternalOutput")
    o_out = nc.dram_tensor("o_out", [ROWS, DLAT], f32, kind="ExternalOutput")

    with TileContext(nc) as tc:
        with (
            tc.tile_pool(name="constp", bufs=1) as constp,
            tc.tile_pool(name="workp", bufs=work_bufs) as workp,
            tc.tile_pool(name="kpool", bufs=2) as kpool,
            tc.tile_pool(name="tsbp", bufs=3) as tsbp,
            tc.tile_pool(name="smallp", bufs=2) as smallp,
            tc.tile_pool(name="psum_o", bufs=2, space="PSUM") as psum_o,
            tc.tile_pool(name="psum_t", bufs=3, space="PSUM") as psum_t,
        ):
            # small consts first so the first chunk's compute can start early
            cst_sb = constp.tile([P, 2], f32, name="cst_sb")
            nc.sync.dma_start(out=cst_sb, in_=consts[:, :])
            ar_sb = constp.tile([P, NRB], f32, name="ar_sb")
            nc.sync.dma_start(out=ar_sb, in_=ar[:, :])
            id_sb = constp.tile([P, P], f32, name="id_sb")
            nc.sync.dma_start(out=id_sb, in_=ident[:, :])
            # big const loads ride the store (ACT) ring, which is idle at
            # startup, so the first D chunks aren't queued behind them;
            # split the 4MB broadcast so chunk 0 unblocks after 1/nch
            b_sb = constp.tile([P, N], f32, name="b_sb")
            for ch in range(nch):
                nc.scalar.dma_start(
                    out=b_sb[:, ch * chunk : (ch + 1) * chunk],
                    in_=bb[:, ch * chunk : (ch + 1) * chunk],
                )
            lat_sb = constp.tile([P, NJB, DL1], bf16, name="lat_sb")
            nc.scalar.dma_start(out=lat_sb, in_=latb[:, :, :])
            eps_ap = cst_sb[:, 0:1]
            zero_ap = cst_sb[:, 1:2]

            # Emission is software-pipelined by one stage: each chunk's
            # PE/copy/matmul block is emitted AFTER the next chunk's
            # elementwise ops, and each row block's normalize/scale/store
            # tail after the next row block's first chunk. Dependencies are
            # unchanged (Tile tracks them); only scheduling priority shifts,
            # keeping the pacing DMA->ACT->DVE chain ahead of consumers.
            pending_pe = None
            pending_tail = None
            for rb in [r for _ in range(repeat) for r in range(NRB)]:
                r0 = rb * P
                k0 = kpool.tile([P, N], f32, name="k0", tag="k0")
                opsum = (
                    psum_o.tile([P, DLAT], f32, name="opsum", tag="op")
                    if do_pe
                    else None
                )
                rs_parts = []
                for ch in range(nch):
                    c0 = ch * chunk
                    if do_elem:
                        dt_ = workp.tile([P, chunk], f32, name="dt", tag="d")
                        nc.sync.dma_start(
                            out=dt_, in_=d[r0 : r0 + P, c0 : c0 + chunk]
                        )
                    else:
                        nc.sync.dma_start(
                            out=k0[:, c0 : c0 + chunk],
                            in_=d[r0 : r0 + P, c0 : c0 + chunk],
                        )
                    if do_elem:
                        lt = workp.tile([P, chunk], f32, name="lt", tag="l")
                        # L = ln(D + 1e-8)
                        nc.scalar.activation(lt, dt_, AF.Ln, bias=eps_ap, scale=1.0)
                        # L = (B + 0.5*alpha_i) * L     (in place over L)
                        nc.vector.scalar_tensor_tensor(
                            lt,
                            b_sb[:, c0 : c0 + chunk],
                            ar_sb[:, rb : rb + 1],
                            lt,
                            op0=OP.add,
                            op1=OP.mult,
                        )
                        # S = D*(1/12) + L              (in place over D)
                        nc.vector.scalar_tensor_tensor(
                            dt_, dt_, 1.0 / 12.0, lt, op0=OP.mult, op1=OP.add
                        )
                        # K0 = exp(-S); accum_out gives the fp32 row-sum free
                        rs = smallp.tile([P, 1], f32, name="rs", tag="rs", bufs=12)
                        nc.scalar.activation(
                            k0[:, c0 : c0 + chunk],
                            dt_,
                            AF.Exp,
                            bias=zero_ap,
                            scale=-1.0,
                            accum_out=rs,
                        )
                        # fold into a running sum now so the row-block
                        # boundary only pays +eps and the reciprocal. These
                        # tiny adds go on the otherwise-idle GPSIMD queue:
                        # on DVE's strict FIFO they would head-of-line block
                        # the next chunk's ready stt behind a wait on exp.
                        if not rs_parts:
                            rs_parts.append(rs)
                        else:
                            rs_run = smallp.tile(
                                [P, 1], f32, name="rs_run", tag="rs", bufs=12
                            )
                            nc.gpsimd.tensor_tensor(
                                rs_run, rs_parts[-1], rs, op=OP.add
                            )
                            rs_parts.append(rs_run)
                    if pending_pe is not None:
                        pending_pe()
                        pending_pe = None
                    if pending_tail is not None:
                        pending_tail()
                        pending_tail = None

                    def pe_block(k0=k0, opsum=opsum, c0=c0):
                        for half in range(chunk // TB):
                            h0 = c0 + half * TB
                            tps = psum_t.tile([P, TB], f32, name="tps", tag="tp")
                            for b8 in range(TB // P):
                                j0 = h0 + b8 * P
                                nc.tensor.transpose(
                                    tps[:, b8 * P : (b8 + 1) * P],
                                    k0[:, j0 : j0 + P],
                                    id_sb,
                                )
                            tsb = tsbp.tile([P, TB], bf16, name="tsb", tag="tsb")
                            nc.scalar.copy(tsb, tps)
                            for b8 in range(TB // P):
                                jb = h0 // P + b8
                                nc.tensor.matmul(
                                    opsum,
                                    tsb[:, b8 * P : (b8 + 1) * P],
                                    lat_sb[:, jb, 0:DLAT],
                                    start=(jb == 0),
                                    stop=(jb == NJB - 1),
                                )

                    if do_pe:
                        pending_pe = pe_block

                def tail_block(rb=rb, r0=r0, k0=k0, opsum=opsum, rs_parts=rs_parts):
                    # inv = 1 / (rowsum + 1e-8)
                    if do_elem and (do_pe or do_scale):
                        inv = smallp.tile([P, 1], f32, name="inv", tag="inv")
                        nc.vector.tensor_scalar_add(inv, rs_parts[-1], 1.0e-8)
                        nc.vector.reciprocal(inv, inv)
                    elif do_scale:
                        inv = smallp.tile([P, 1], f32, name="inv", tag="inv")
                        nc.vector.memset(inv, 1.0)
                    if do_pe:
                        osb = smallp.tile([P, DLAT], f32, name="osb", tag="osb")
                        nc.vector.tensor_scalar_mul(osb, opsum[:, 0:DLAT], inv)
                        nc.sync.dma_start(out=o_out[r0 : r0 + P, :], in_=osb)
                    for ch2 in range(nch):
                        c2 = ch2 * chunk
                        if do_scale:
                            nc.vector.tensor_scalar_mul(
                                k0[:, c2 : c2 + chunk], k0[:, c2 : c2 + chunk], inv
                            )
                        nc.scalar.dma_start(
                            out=k_out[r0 : r0 + P, c2 : c2 + chunk],
                            in_=k0[:, c2 : c2 + chunk],
                        )

                # the tail needs the last chunk's matmuls emitted first
                if pending_pe is not None:
                    pending_pe()
                    pending_pe = None
                pending_tail = tail_block
            if pending_tail is not None:
                pending_tail()
                pending_tail = None

    _compile_with_shared_act_set(nc)
    return nc


def get_nc():
    if "nc" not in _CACHE:
        _CACHE["nc"] = _build_nc()
    return _CACHE["nc"]


def prepare_in_maps(latent, D, alpha):
    latent = np.asarray(latent, dtype=np.float32)
    D = np.asarray(D, dtype=np.float32)
    alpha = np.asarray(alpha, dtype=np.float32)

    import ml_dtypes

    lat_aug = np.concatenate([latent, np.ones((N, 1), np.float32)], axis=1)  # [N, 65]
    # lat_blocks[p, jb, c] = lat_aug[jb*128 + p, c]
    lat_blocks = np.ascontiguousarray(
        lat_aug.reshape(NJB, P, DL1).transpose(1, 0, 2)
    ).astype(ml_dtypes.bfloat16)
    b_bcast = np.ascontiguousarray(
        np.broadcast_to(0.5 * alpha[None, :], (P, N))
    )
    ident = np.eye(P, dtype=np.float32)
    consts = np.zeros((P, 2), np.float32)
    consts[:, 0] = 1.0e-8

    in_maps = []
    for c in range(NCORES):
        rows = slice(c * ROWS, (c + 1) * ROWS)
        d_shard = D[rows].copy()
        d_shard[np.arange(ROWS), np.arange(c * ROWS, (c + 1) * ROWS)] = DIAG_POISON
        # alpha_rows[p, rb] = 0.5*alpha[c*ROWS + rb*128 + p]
        alpha_rows = np.ascontiguousarray(
            (0.5 * alpha[rows]).reshape(NRB, P).T
        )
        in_maps.append(
            {
                "d_shard": d_shard,
                "b_bcast": b_bcast,
                "lat_blocks": lat_blocks,
                "alpha_rows": alpha_rows,
                "ident": ident,
                "consts": consts,
            }
        )
    return in_maps


def assemble(results):
    K = np.concatenate([r["k_out"] for r in results], axis=0)
    out = np.concatenate([r["o_out"] for r in results], axis=0)
    return out, K


def kernel(latent, D, alpha):
    from concourse.bass_utils import run_bass_kernel_spmd

    nc = get_nc()
    in_maps = prepare_in_maps(latent, D, alpha)
    res = run_bass_kernel_spmd(nc, in_maps, core_ids=list(range(NCORES)))
    return assemble(res.results)
